# revision 1
# baseline (speedup 1.0000x reference)
"""Multi-head attention (B=2, S=2048, D=1024, H=16) on 8 TRN2 NeuronCores.

Sharding: core = (batch b, head-group g): 2 batches x 4 groups of 4 heads.
Each core computes its group's QKV projections, attention, and a partial
output projection; the host sums the 4 partials per batch and adds the
exact bias constant (bv @ Wo.T + bo). bq/bk are applied on device.

Matmul dtype is configurable (default fp16):
  fp16: operands stored/shipped as float16 (10-bit mantissa), 1 PE
        cycle/row + fast weight load; fp32 PSUM accumulation.
        ~1.3e-3 max rel error at the same speed as bf16.
  bf16: same rate, ~8e-3 max rel error.
  f32r: fp32 data rounded to the PE's TF32-like fast format, 2 cycles/row.
        ~5.5e-4 max rel error, ~1.5x slower.
The softmax normalization chain stays in f32/f32r in every mode so the
denominator carries no 16-bit error.

Per-core layout:
  xT [D, S] host-transposed inputs; QT/KT [JJ, S] head-dim-major so scores
  come out keys-on-partitions (S.T tiles) and the key-axis softmax reduction
  happens inside the P.T @ V' matmul via a ones-column appended to V'
  (PSUM row 64 of the PV output accumulates the softmax denominator).
  V' stationaries are padded to 128 columns to keep fast weight loads.
  OT [JJ, S] normalized attention output feeds the output projection as
  lhsT, giving the partial output in natural [S, D] layout.
"""
from contextlib import ExitStack

import numpy as np

# Problem constants (hardcoded per harness contract).
B, S, D, H = 2, 2048, 1024, 16
HD = D // H          # 64
N_CORES = 8
GROUPS = N_CORES // B    # 4
H_LOC = H // GROUPS      # 4 heads per core
JJ = H_LOC * HD          # 256
P = 128

MM_DT = "fp16"  # "fp16" | "bf16" | "f32r"


def build_mha(s=S, d=D, h_loc=H_LOC, hd=HD, chunk=1024, nf=512, mm_dt=MM_DT):
    """Build + compile the per-core Bass program."""
    import concourse.bacc as bacc
    import concourse.tile as tile
    from concourse import mybir

    f32 = mybir.dt.float32
    f32r = mybir.dt.float32r
    _two_byte = {"bf16": mybir.dt.bfloat16, "fp16": mybir.dt.float16}
    mdt = _two_byte.get(mm_dt, f32r)
    in_dt = _two_byte.get(mm_dt, f32)  # DRAM dtype of x / weights
    Exp = mybir.ActivationFunctionType.Exp
    Ident = mybir.ActivationFunctionType.Identity

    jj = h_loc * hd
    hd1 = hd + 1
    ktd = d // P
    njt = (jj + P - 1) // P
    st_n = s // P
    chunk = min(chunk, s)
    nf = min(nf, chunk)
    n_ch = s // chunk
    nfc = chunk // nf
    ndo = (d + nf - 1) // nf
    pc = min(512, s)

    nc = bacc.Bacc("TRN2", target_bir_lowering=False, debug=False)

    xq = nc.dram_tensor("xq", [d, s], in_dt, kind="ExternalInput").ap()
    xk = nc.dram_tensor("xk", [d, s], in_dt, kind="ExternalInput").ap()
    xv = nc.dram_tensor("xv", [d, s], in_dt, kind="ExternalInput").ap()
    wq = nc.dram_tensor("wq", [d, jj], in_dt, kind="ExternalInput").ap()
    wk = nc.dram_tensor("wk", [d, jj], in_dt, kind="ExternalInput").ap()
    wv = nc.dram_tensor("wv", [d, jj], in_dt, kind="ExternalInput").ap()
    wo = nc.dram_tensor("wo", [jj, d], in_dt, kind="ExternalInput").ap()
    bqp = nc.dram_tensor("bqp", [jj, 1], f32, kind="ExternalInput").ap()
    bkp = nc.dram_tensor("bkp", [jj, 1], f32, kind="ExternalInput").ap()
    out = nc.dram_tensor("out", [s, d], f32, kind="ExternalOutput").ap()

    with tile.TileContext(nc) as tc, ExitStack() as ctx:
        persist = ctx.enter_context(tc.tile_pool(name="persist", bufs=1))

        qt_sb = [persist.tile([P, s], mdt, name=f"qt{j}", tag=f"qt{j}") for j in range(njt)]
        kt_sb = [persist.tile([P, s], mdt, name=f"kt{j}", tag=f"kt{j}") for j in range(njt)]
        ot_sb = [persist.tile([P, s], mdt, name=f"ot{j}", tag=f"ot{j}") for j in range(njt)]
        # padded per-(seq-tile, head) PV stationaries: [V_h | ones | zeros]
        v_sb = [[persist.tile([P, P], mdt, name=f"v{t}_{h}", tag=f"v{t}_{h}")
                 for h in range(h_loc)] for t in range(st_n)]
        wq_r = [persist.tile([P, jj], mdt, name=f"wqr{k}", tag=f"wqr{k}") for k in range(ktd)]
        wk_r = [persist.tile([P, jj], mdt, name=f"wkr{k}", tag=f"wkr{k}") for k in range(ktd)]
        wv_r = [persist.tile([P, jj], mdt, name=f"wvr{k}", tag=f"wvr{k}") for k in range(ktd)]
        wo_r = [persist.tile([P, d], mdt, name=f"wor{j}", tag=f"wor{j}") for j in range(njt)]
        bq_sb = persist.tile([P, njt], f32, name="bq_sb", tag="bq_sb")
        bk_sb = persist.tile([P, njt], f32, name="bk_sb", tag="bk_sb")
        ones_v = persist.tile([P, 1], f32, name="ones_v", tag="ones_v")
        ones_h = persist.tile([1, hd], f32, name="ones_h", tag="ones_h")
        ones_hr = persist.tile([1, hd], f32r, name="ones_hr", tag="ones_hr")

        nc.vector.memset(ones_v[:], 1.0)
        nc.vector.memset(ones_h[:], 1.0)
        nc.vector.tensor_copy(ones_hr[:], ones_h[:])
        for j in range(njt):
            nc.scalar.dma_start(bq_sb[:, j:j + 1], bqp[j * P:(j + 1) * P, :])
            nc.scalar.dma_start(bk_sb[:, j:j + 1], bkp[j * P:(j + 1) * P, :])

        # ---- weights ----
        # wk first: the K-projection is the first consumer, so its weight
        # tiles and xk tiles (loaded right after, in load_xr) lead the DMA
        # queue; wq/wv/wo follow on the scalar-engine HWDGE queue so they
        # don't delay the critical path.
        if mm_dt in _two_byte:
            for k in range(ktd):
                nc.sync.dma_start(wk_r[k][:], wk[k * P:(k + 1) * P, :])
            for k in range(ktd):
                nc.scalar.dma_start(wq_r[k][:], wq[k * P:(k + 1) * P, :])
                nc.scalar.dma_start(wv_r[k][:], wv[k * P:(k + 1) * P, :])
            for j in range(njt):
                nc.scalar.dma_start(wo_r[j][:], wo[j * P:(j + 1) * P, :])
        else:
            with tc.tile_pool(name="wstage", bufs=3) as wstage:
                for k in range(ktd):
                    for nm, dr, dst in (("q", wq, wq_r), ("k", wk, wk_r), ("v", wv, wv_r)):
                        wtmp = wstage.tile([P, jj], f32, name=f"w{nm}s{k}", tag="wst")
                        nc.sync.dma_start(wtmp[:], dr[k * P:(k + 1) * P, :])
                        nc.vector.tensor_copy(dst[k][:], wtmp[:])
                for j in range(njt):
                    wtmp = wstage.tile([P, d], f32, name=f"wos{j}", tag="wost")
                    nc.sync.dma_start(wtmp[:], wo[j * P:(j + 1) * P, :])
                    nc.vector.tensor_copy(wo_r[j][:], wtmp[:])

        # ---- projections ----
        with tc.tile_pool(name="xpool", bufs=3) as xpool, \
             tc.tile_pool(name="xrpool", bufs=ktd) as xrpool, \
             tc.tile_pool(name="ppsum", bufs=3, space="PSUM") as ppsum:

            def load_xr(xdr):
                tiles = []
                for k in range(ktd):
                    if mm_dt in ("bf16", "fp16"):
                        xr = xrpool.tile([P, s], mdt, name=f"xr{k}", tag="xr")
                        nc.sync.dma_start(xr[:], xdr[k * P:(k + 1) * P, :])
                    else:
                        xs = xpool.tile([P, s], f32, name=f"xs{k}", tag="xs")
                        nc.sync.dma_start(xs[:], xdr[k * P:(k + 1) * P, :])
                        xr = xrpool.tile([P, s], mdt, name=f"xr{k}", tag="xr")
                        nc.vector.tensor_copy(xr[:], xs[:])
                    tiles.append(xr)
                return tiles

            for nm, xdr, w_r, dst, bias_sb, scale in (
                ("k", xk, wk_r, kt_sb, bk_sb, 1.0),
                ("q", xq, wq_r, qt_sb, bq_sb, float(1.0 / np.sqrt(hd))),
            ):
                xr_t = load_xr(xdr)
                ncp = s // pc
                for j in range(njt):
                    # k-outer / c-inner so each weight stationary load serves
                    # ncp moving streams
                    pps = [ppsum.tile([P, pc], f32, name=f"pp{nm}{j}_{c}", tag="pp",
                                      bufs=ncp + 1)
                           for c in range(ncp)]
                    for k in range(ktd):
                        for c in range(ncp):
                            nc.tensor.matmul(
                                pps[c][:], w_r[k][:, j * P:(j + 1) * P],
                                xr_t[k][:, c * pc:(c + 1) * pc],
                                start=(k == 0), stop=(k == ktd - 1))
                    for c in range(ncp):
                        nc.scalar.activation(
                            dst[j][:, c * pc:(c + 1) * pc], pps[c][:], Ident,
                            bias=bias_sb[:, j:j + 1], scale=scale)

            # V' padded stationaries
            xr_t = load_xr(xv)
            for t in range(st_n):
                pv = ppsum.tile([P, jj], f32, name=f"pv{t}", tag="pv", bufs=3)
                for k in range(ktd):
                    nc.tensor.matmul(pv[:], xr_t[k][:, t * P:(t + 1) * P],
                                     wv_r[k][:], start=(k == 0), stop=(k == ktd - 1))
                for h in range(h_loc):
                    vt = v_sb[t][h]
                    nc.vector.tensor_copy(vt[:, 0:hd], pv[:, h * hd:(h + 1) * hd])
                    nc.vector.tensor_copy(vt[:, hd:hd1], ones_v[:])
                    if hd1 < P:
                        nc.gpsimd.memset(vt[:, hd1:P], 0.0)

        # ---- attention ----
        # Per head, two passes over the full sequence:
        #   pass 1: scores.T tiles (one KT stationary load per seq-tile, s/nf
        #           moving streams) -> exp over [128, s] -> PT tiles
        #   pass 2: PV accumulation (one V' stationary load per seq-tile,
        #           s/nf moving streams) -> [128, s] psum, row hd = denominators
        # PSUM: sp [128,s] (s/512 banks) + otp [128,s] -> 8 banks total.
        ec = min(1024, s)          # exp / score-psum chunk of the q axis
        nec = s // ec
        efc = ec // nf
        with tc.tile_pool(name="spsum", bufs=2, space="PSUM") as spsum, \
             tc.tile_pool(name="opsum", bufs=1, space="PSUM") as opsum, \
             tc.tile_pool(name="ptpool", bufs=3 * nec + 2) as ptpool, \
             tc.tile_pool(name="npool", bufs=2) as npool:
            pending_norm = None
            for h in range(h_loc):
                jt = (h * hd) // P
                off = (h * hd) % P
                otp = opsum.tile([P, s], f32, name=f"otp{h}", tag="otp")
                pts = {}

                def scores(t):
                    for e in range(nec):
                        sp = spsum.tile([P, ec], f32, name=f"sp{h}_{t}_{e}", tag="sp")
                        for f in range(efc):
                            q0 = e * ec + f * nf
                            nc.tensor.matmul(
                                sp[:, f * nf:(f + 1) * nf],
                                kt_sb[jt][off:off + hd, t * P:(t + 1) * P],
                                qt_sb[jt][off:off + hd, q0:q0 + nf],
                                start=True, stop=True)
                        pt = ptpool.tile([P, ec], mdt, name=f"pt{h}_{t}_{e}", tag="pt")
                        nc.scalar.activation(pt[:], sp[:], Exp)
                        pts[t, e] = pt

                def pv(t):
                    for e in range(nec):
                        for f in range(efc):
                            q0 = e * ec + f * nf
                            nc.tensor.matmul(
                                otp[:, q0:q0 + nf],
                                v_sb[t][h][:],
                                pts[t, e][:, f * nf:(f + 1) * nf],
                                start=(t == 0), stop=(t == st_n - 1))
                        del pts[t, e]

                # software-pipeline: scores(t+1) emitted before pv(t); the
                # previous head's normalize is emitted into this head's
                # scores stream so its broadcast matmuls don't stall the PE.
                scores(0)
                for t in range(1, st_n):
                    scores(t)
                    if t == 2 and pending_norm is not None:
                        pending_norm()
                        pending_norm = None
                    pv(t - 1)
                pv(st_n - 1)
                # Evict the PV accumulator to SBUF with one copy so the PSUM
                # frees for the next head immediately; the normalize chain
                # (rowsum broadcast, reciprocal, scale) runs off the critical
                # path, chunked so the output projection can start early.
                rs_r = npool.tile([1, s], f32r, name=f"rs{h}", tag="rs")
                nc.scalar.activation(rs_r[:], otp[hd:hd1, :],
                                     mybir.ActivationFunctionType.Copy)
                ob = npool.tile([hd, s], f32, name=f"obuf{h}", tag="obuf")
                nc.vector.tensor_copy(ob[:], otp[0:hd, :])

                # the last head's normalize gates the output projection, so
                # chunk it finer there to release early columns sooner
                cw = nf if h == h_loc - 1 else ec
                cfc = cw // nf

                def norm(ob=ob, rs_r=rs_r, jt=jt, off=off, h=h, cw=cw, cfc=cfc):
                    for e in range(s // cw):
                        bp = spsum.tile([hd, cw], f32, name=f"bp{h}_{e}", tag="sp")
                        for f in range(cfc):
                            q0 = e * cw + f * nf
                            nc.tensor.matmul(bp[:, f * nf:(f + 1) * nf],
                                             ones_hr[:], rs_r[:, q0:q0 + nf],
                                             start=True, stop=True)
                        binv = npool.tile([hd, cw], f32,
                                          name=f"binv{h}_{e}", tag="binv")
                        nc.vector.reciprocal(binv[:], bp[:])
                        nc.vector.tensor_mul(
                            ot_sb[jt][off:off + hd, e * cw:(e + 1) * cw],
                            ob[0:hd, e * cw:(e + 1) * cw], binv[:])

                if pending_norm is not None:  # small-config fallback
                    pending_norm()
                pending_norm = norm
            pending_norm()

        # ---- output projection (natural layout) ----
        with tc.tile_pool(name="fpsum", bufs=2, space="PSUM") as fpsum, \
             tc.tile_pool(name="fout", bufs=2) as fout:
            for t in range(st_n):
                po = fpsum.tile([P, d], f32, name=f"po{t}", tag="po")
                for njx in range(ndo):
                    for j in range(njt):
                        nc.tensor.matmul(
                            po[:, njx * nf:(njx + 1) * nf],
                            ot_sb[j][:, t * P:(t + 1) * P],
                            wo_r[j][:, njx * nf:(njx + 1) * nf],
                            start=(j == 0), stop=(j == njt - 1))
                ob = fout.tile([P, d], f32, name=f"ob{t}", tag="ob")
                nc.scalar.copy(ob[:], po[:])
                nc.sync.dma_start(out[t * P:(t + 1) * P, :], ob[:])

    nc.compile()
    return nc


_NC_CACHE = {}


def _get_nc():
    key = MM_DT
    if key not in _NC_CACHE:
        _NC_CACHE[key] = build_mha(mm_dt=key)
    return _NC_CACHE[key]


def build_in_maps(inputs, mm_dt=MM_DT):
    if mm_dt == "bf16":
        import ml_dtypes
        xdt = ml_dtypes.bfloat16
    elif mm_dt == "fp16":
        xdt = np.float16
    else:
        xdt = np.float32

    q = np.asarray(inputs["query"], np.float32)
    k = np.asarray(inputs.get("key_", inputs.get("key")), np.float32)
    v = np.asarray(inputs["value"], np.float32)
    Wq = np.asarray(inputs["Wq"], np.float32)
    Wk = np.asarray(inputs["Wk"], np.float32)
    Wv = np.asarray(inputs["Wv"], np.float32)
    Wo = np.asarray(inputs["Wo"], np.float32)
    bq = np.asarray(inputs["bq"], np.float32)
    bk = np.asarray(inputs["bk"], np.float32)

    sc = np.float32(1.0 / np.sqrt(HD))
    qT = [np.ascontiguousarray(q[b].T).astype(xdt) for b in range(B)]
    kT = [np.ascontiguousarray(k[b].T).astype(xdt) for b in range(B)]
    vT = [np.ascontiguousarray(v[b].T).astype(xdt) for b in range(B)]
    WqT = np.ascontiguousarray(Wq.T)
    WkT = np.ascontiguousarray(Wk.T)
    WvT = np.ascontiguousarray(Wv.T)

    in_maps = []
    for core in range(N_CORES):
        b, g = divmod(core, GROUPS)
        sl = slice(g * JJ, (g + 1) * JJ)
        in_maps.append({
            "xq": qT[b],
            "xk": kT[b],
            "xv": vT[b],
            "wq": np.ascontiguousarray(WqT[:, sl]).astype(xdt),
            "wk": np.ascontiguousarray(WkT[:, sl]).astype(xdt),
            "wv": np.ascontiguousarray(WvT[:, sl]).astype(xdt),
            "wo": np.ascontiguousarray(Wo[:, sl].T).astype(xdt),
            "bqp": np.ascontiguousarray((bq[sl] * sc)[:, None]),
            "bkp": np.ascontiguousarray(bk[sl][:, None]),
        })
    return in_maps


def combine_outputs(results, inputs):
    Wo = np.asarray(inputs["Wo"], np.float32)
    bv = np.asarray(inputs["bv"], np.float32)
    bo = np.asarray(inputs["bo"], np.float32)
    const = bv @ Wo.T + bo  # exact host-side bias correction
    outp = np.empty((B, S, D), np.float32)
    for b in range(B):
        acc = results[b * GROUPS]["out"].astype(np.float32).copy()
        for g in range(1, GROUPS):
            acc += results[b * GROUPS + g]["out"]
        outp[b] = acc + const[None, :]
    return outp


def kernel(**inputs):
    import time
    from concourse.bass_utils import run_bass_kernel_spmd

    nc = _get_nc()
    in_maps = build_in_maps(inputs)
    last_err = None
    for attempt in range(3):
        try:
            res = run_bass_kernel_spmd(nc, in_maps, list(range(N_CORES)))
            return combine_outputs(res.results, inputs)
        except Exception as e:  # transient device wedge: retry
            last_err = e
            try:
                # poke each core with a trivial op to clear transient
                # exec-unit state before retrying
                import jax
                import jax.numpy as jnp
                for dvc in jax.devices()[:N_CORES]:
                    jax.device_put(jnp.zeros((8, 8)), dvc).block_until_ready()
            except Exception:
                pass
            time.sleep(5.0 * (attempt + 1))
    raise last_err



# revision 13
# speedup vs baseline: 1.5532x; 1.5532x over previous
"""Multi-head attention (B=2, S=2048, D=1024, H=16) on 8 TRN2 NeuronCores.

Sharding: core = (batch b, head-group g): 2 batches x 4 groups of 4 heads.
Each core computes its group's QKV projections, attention, and a partial
output projection; the host sums the 4 partials per batch and adds the
exact bias constant (bv @ Wo.T + bo). bq/bk are applied on device.

Engine budget per core (warm PE @2.4GHz): PE ~165us of matmul columns,
ACT ~142us of exp, DVE ~40us of evac/normalize. The kernel is structured
so the PE never idles long enough for the HAM clock gate to re-throttle:

  * every matmul presents a full 128-row stationary to the array. The
    hd=64 score matmuls are padded with explicit zero rows (per-head KT
    tiles [128, s] with zeros outside the head's 64 rows) so the padded
    rows multiply the other head's moving data by 0.0 -- same cycle
    count, full array activity.
  * projections run k-outer so each arriving x-tile is consumed once,
    back-to-back; all input DMAs are issued upfront on one queue in
    consumption order (xk, xv, xq).
  * a short burst of dummy matmuls warms the PE during the initial DMA
    window, and a dummy exp preloads the ACT table set.
  * ACT runs exp only. Projection bias+scale, PSUM evacuations, and the
    softmax normalization run on DVE (reciprocal_approx_fast on the [1,s]
    denominator row, then a PE broadcast matmul of the reciprocal).

Per-core layout:
  xT [D, S] host-transposed inputs; QT [128, S] pair-packed, KT [128, S]
  per-head zero-padded, head-dim-major so scores come out keys-on-
  partitions; the key-axis softmax reduction happens inside the P.T @ V'
  matmul via a ones-column appended to V' (PSUM row 64 of the PV output
  accumulates the softmax denominator). OT [128, S] pair-packed feeds the
  output projection as lhsT, giving the partial output in natural [S, D]
  layout, written back as fp16 (host upcasts and combines).
"""
from contextlib import ExitStack

import numpy as np

# Problem constants (hardcoded per harness contract).
B, S, D, H = 2, 2048, 1024, 16
HD = D // H          # 64
N_CORES = 8
GROUPS = N_CORES // B    # 4
H_LOC = H // GROUPS      # 4 heads per core
JJ = H_LOC * HD          # 256
P = 128

MM_DT = "fp16"  # "fp16" | "bf16"


def build_mha(s=S, d=D, h_loc=H_LOC, hd=HD, chunk=1024, nf=512, mm_dt=MM_DT,
              dbg=False):
    """Build + compile the per-core Bass program."""
    import concourse.bacc as bacc
    import concourse.tile as tile
    from concourse import mybir

    f32 = mybir.dt.float32
    _two_byte = {"bf16": mybir.dt.bfloat16, "fp16": mybir.dt.float16}
    assert mm_dt in _two_byte
    mdt = _two_byte[mm_dt]
    in_dt = mdt
    Exp = mybir.ActivationFunctionType.Exp
    MULT = mybir.AluOpType.mult
    ADD = mybir.AluOpType.add

    jj = h_loc * hd
    hd1 = hd + 1
    ktd = d // P
    njt = (jj + P - 1) // P
    st_n = s // P
    nf = min(nf, s)
    nfc = s // nf            # moving chunks per full row
    ndo = (d + nf - 1) // nf

    nc = bacc.Bacc("TRN2", target_bir_lowering=False, debug=False)

    xq = nc.dram_tensor("xq", [d, s], in_dt, kind="ExternalInput").ap()
    xk = nc.dram_tensor("xk", [d, s], in_dt, kind="ExternalInput").ap()
    xv = nc.dram_tensor("xv", [d, s], in_dt, kind="ExternalInput").ap()
    wq = nc.dram_tensor("wq", [d, jj], in_dt, kind="ExternalInput").ap()
    wk = nc.dram_tensor("wk", [d, jj], in_dt, kind="ExternalInput").ap()
    wv = nc.dram_tensor("wv", [d, jj], in_dt, kind="ExternalInput").ap()
    wo = nc.dram_tensor("wo", [jj, d], in_dt, kind="ExternalInput").ap()
    bqp = nc.dram_tensor("bqp", [jj, 1], f32, kind="ExternalInput").ap()
    bkp = nc.dram_tensor("bkp", [jj, 1], f32, kind="ExternalInput").ap()
    out = nc.dram_tensor("out", [s, d], mdt, kind="ExternalOutput").ap()
    if dbg:
        dq = nc.dram_tensor("dq", [P, s], mdt, kind="ExternalOutput").ap()
        dk = nc.dram_tensor("dk", [P, s], mdt, kind="ExternalOutput").ap()
        dv = nc.dram_tensor("dv", [P, P], mdt, kind="ExternalOutput").ap()
        dpt = nc.dram_tensor("dpt", [P, min(1024, s)], mdt, kind="ExternalOutput").ap()
        dob = nc.dram_tensor("dob", [hd, s], f32, kind="ExternalOutput").ap()
        drs = nc.dram_tensor("drs", [2, s], mdt, kind="ExternalOutput").ap()
        dot = nc.dram_tensor("dot", [P, s], mdt, kind="ExternalOutput").ap()

    with tile.TileContext(nc) as tc, ExitStack() as ctx:
        persist = ctx.enter_context(tc.tile_pool(name="persist", bufs=1))

        qt_sb = [persist.tile([P, s], mdt, name=f"qt{j}", tag=f"qt{j}") for j in range(njt)]
        # per-head KT, zero rows outside the head's hd slice (full-row scores)
        kt_sb = [persist.tile([P, s], mdt, name=f"kt{h}", tag=f"kt{h}") for h in range(h_loc)]
        ot_sb = [persist.tile([P, s], mdt, name=f"ot{j}", tag=f"ot{j}") for j in range(njt)]
        # padded per-(seq-tile, head) PV stationaries: [V_h | ones | zeros]
        v_sb = [[persist.tile([P, P], mdt, name=f"v{t}_{h}", tag=f"v{t}_{h}")
                 for h in range(h_loc)] for t in range(st_n)]
        wq_r = [persist.tile([P, jj], mdt, name=f"wqr{k}", tag=f"wqr{k}") for k in range(ktd)]
        wk_r = [persist.tile([P, jj], mdt, name=f"wkr{k}", tag=f"wkr{k}") for k in range(ktd)]
        wv_r = [persist.tile([P, jj], mdt, name=f"wvr{k}", tag=f"wvr{k}") for k in range(ktd)]
        wo_r = [persist.tile([P, d], mdt, name=f"wor{j}", tag=f"wor{j}") for j in range(njt)]
        bq_sb = persist.tile([P, njt], f32, name="bq_sb", tag="bq_sb")
        bk_sb = persist.tile([P, njt], f32, name="bk_sb", tag="bk_sb")
        ones_v = persist.tile([P, 1], f32, name="ones_v", tag="ones_v")
        # norm broadcast: stationary row0=1 rest 0; moving row0=recip(denom)
        ones_bc = persist.tile([P, hd], mdt, name="ones_bc", tag="ones_bc")
        rs128 = [persist.tile([P, s], mdt, name=f"rs{i}", tag=f"rs{i}")
                 for i in range(2)]
        wm_a = persist.tile([P, nf], mdt, name="wm_a", tag="wm_a")
        ep_t = persist.tile([1, 8], f32, name="ep_t", tag="ep_t")

        # ---- preamble: exp-table preload, PE warmup, zero padding ----
        nc.vector.memset(ep_t[:], 0.0)
        nc.scalar.activation(ep_t[:], ep_t[:], Exp)  # pulls ACT table load early
        nc.vector.memset(ones_v[:], 1.0)
        nc.vector.memset(wm_a[:], 0.0)
        nc.gpsimd.memset(ones_bc[:], 0.0)
        nc.gpsimd.memset(ones_bc[0:1, :], 1.0)
        for i in range(2):
            nc.gpsimd.memset(rs128[i][:], 0.0)
        for h in range(h_loc):
            off = (h * hd) % P
            if off > 0:
                nc.gpsimd.memset(kt_sb[h][0:off, :], 0.0)
            if off + hd < P:
                nc.gpsimd.memset(kt_sb[h][off + hd:P, :], 0.0)
        for j in range(njt):
            nc.scalar.dma_start(bq_sb[:, j:j + 1], bqp[j * P:(j + 1) * P, :])
            nc.scalar.dma_start(bk_sb[:, j:j + 1], bkp[j * P:(j + 1) * P, :])

        # ---- weights ----
        # wk on the sync queue ahead of the x tensors (first consumer);
        # everything else on the scalar-engine HWDGE queue.
        for k in range(ktd):
            nc.sync.dma_start(wk_r[k][:], wk[k * P:(k + 1) * P, :])
        for k in range(ktd):
            nc.scalar.dma_start(wv_r[k][:], wv[k * P:(k + 1) * P, :])
        for k in range(ktd):
            nc.scalar.dma_start(wq_r[k][:], wq[k * P:(k + 1) * P, :])
        for j in range(njt):
            nc.scalar.dma_start(wo_r[j][:], wo[j * P:(j + 1) * P, :])

        # PE warmup burst (runs while the first x tiles stream in)
        with tc.tile_pool(name="wup", bufs=1, space="PSUM") as wup:
            wm_p = wup.tile([P, nf], f32, name="wm_p", tag="wm_p")
            for i in range(10):
                nc.tensor.matmul(wm_p[:], wm_a[:, 0:P], wm_a[:], start=True, stop=True)
            # token reader so the warmup matmuls can't be elided
            nc.vector.tensor_copy(ep_t[0:1, 0:8], wm_p[0:1, 0:8])

        # ---- projections (K, V, Q; k-outer so each x tile is consumed once) ----
        with tc.tile_pool(name="xrpool", bufs=12) as xrpool:
            def load_xr(xdr, nm):
                tiles = []
                for k in range(ktd):
                    xr = xrpool.tile([P, s], mdt, name=f"x{nm}{k}", tag="xr")
                    nc.sync.dma_start(xr[:], xdr[k * P:(k + 1) * P, :])
                    tiles.append(xr)
                return tiles

            xk_t = load_xr(xk, "k")
            xv_t = load_xr(xv, "v")
            xq_t = load_xr(xq, "q")

            # K projection -> per-head zero-padded KT
            with tc.tile_pool(name="kpsum", bufs=1, space="PSUM") as kpsum:
                ppk = [kpsum.tile([P, s], f32, name=f"ppk{j}", tag=f"ppj{j}")
                       for j in range(njt)]
                for k in range(ktd):
                    for j in range(njt):
                        for c in range(nfc):
                            nc.tensor.matmul(
                                ppk[j][:, c * nf:(c + 1) * nf],
                                wk_r[k][:, j * P:(j + 1) * P],
                                xk_t[k][:, c * nf:(c + 1) * nf],
                                start=(k == 0), stop=(k == ktd - 1))
                for j in range(njt):
                    for hh in range(P // hd):
                        h = j * (P // hd) + hh
                        if h >= h_loc:
                            continue
                        r0 = hh * hd
                        for c in range(nfc):
                            nc.vector.tensor_scalar(
                                kt_sb[h][r0:r0 + hd, c * nf:(c + 1) * nf],
                                ppk[j][r0:r0 + hd, c * nf:(c + 1) * nf],
                                1.0, bk_sb[r0:r0 + hd, j:j + 1],
                                op0=MULT, op1=ADD)

            # V projection -> padded PV stationaries
            with tc.tile_pool(name="vpsum", bufs=1, space="PSUM") as vpsum:
                for t in range(st_n):
                    pv = vpsum.tile([P, jj], f32, name=f"pv{t}", tag="pv", bufs=3)
                    for k in range(ktd):
                        nc.tensor.matmul(pv[:], xv_t[k][:, t * P:(t + 1) * P],
                                         wv_r[k][:], start=(k == 0), stop=(k == ktd - 1))
                    for h in range(h_loc):
                        vt = v_sb[t][h]
                        nc.vector.tensor_copy(vt[:, 0:hd], pv[:, h * hd:(h + 1) * hd])
                        nc.vector.tensor_copy(vt[:, hd:hd1], ones_v[:])
                        if hd1 < P:
                            nc.gpsimd.memset(vt[:, hd1:P], 0.0)

            # Q projection -> pair-packed QT (scale folded via DVE)
            sc = float(1.0 / np.sqrt(hd))
            with tc.tile_pool(name="qpsum", bufs=1, space="PSUM") as qpsum:
                ppq = [qpsum.tile([P, s], f32, name=f"ppq{j}", tag=f"ppj{j}")
                       for j in range(njt)]
                for k in range(ktd):
                    for j in range(njt):
                        for c in range(nfc):
                            nc.tensor.matmul(
                                ppq[j][:, c * nf:(c + 1) * nf],
                                wq_r[k][:, j * P:(j + 1) * P],
                                xq_t[k][:, c * nf:(c + 1) * nf],
                                start=(k == 0), stop=(k == ktd - 1))
                for j in range(njt):
                    for c in range(nfc):
                        nc.vector.tensor_scalar(
                            qt_sb[j][:, c * nf:(c + 1) * nf],
                            ppq[j][:, c * nf:(c + 1) * nf],
                            sc, bq_sb[:, j:j + 1],
                            op0=MULT, op1=ADD)

        if dbg:
            nc.gpsimd.dma_start(dq[:], qt_sb[0][:])
            nc.gpsimd.dma_start(dk[:], kt_sb[0][:])
            nc.gpsimd.dma_start(dv[:], v_sb[0][0][:])

        # ---- attention ----
        # Per head, two passes over the full sequence:
        #   pass 1: scores.T tiles (full-row stationary from the padded KT)
        #           -> exp over [128, s] -> PT tiles
        #   pass 2: PV accumulation -> [128, s] psum, row hd = denominators
        # PSUM: sp [128, ec] x2 (4 banks) + otp [128, s] (4 banks).
        ec = min(1024, s)          # exp / score-psum chunk of the q axis
        nec = s // ec
        efc = ec // nf
        with tc.tile_pool(name="spsum", bufs=2, space="PSUM") as spsum, \
             tc.tile_pool(name="opsum", bufs=1, space="PSUM") as opsum, \
             tc.tile_pool(name="ptpool", bufs=3 * nec + 2) as ptpool, \
             tc.tile_pool(name="npool", bufs=2) as npool:
            pending_norm = None
            for h in range(h_loc):
                jt = (h * hd) // P
                off = (h * hd) % P
                otp = opsum.tile([P, s], f32, name=f"otp{h}", tag="otp")
                pts = {}

                def scores(t):
                    for e in range(nec):
                        sp = spsum.tile([P, ec], f32, name=f"sp{h}_{t}_{e}", tag="sp")
                        for f in range(efc):
                            q0 = e * ec + f * nf
                            nc.tensor.matmul(
                                sp[:, f * nf:(f + 1) * nf],
                                kt_sb[h][:, t * P:(t + 1) * P],
                                qt_sb[jt][:, q0:q0 + nf],
                                start=True, stop=True)
                        pt = ptpool.tile([P, ec], mdt, name=f"pt{h}_{t}_{e}", tag="pt")
                        nc.scalar.activation(pt[:], sp[:], Exp)
                        if dbg and h == 0 and t == 0 and e == 0:
                            nc.gpsimd.dma_start(dpt[:, 0:ec], pt[:])
                        pts[t, e] = pt

                def pv(t):
                    for e in range(nec):
                        for f in range(efc):
                            q0 = e * ec + f * nf
                            nc.tensor.matmul(
                                otp[:, q0:q0 + nf],
                                v_sb[t][h][:],
                                pts[t, e][:, f * nf:(f + 1) * nf],
                                start=(t == 0), stop=(t == st_n - 1))
                        del pts[t, e]

                # software-pipeline: scores(t+1) emitted before pv(t); the
                # previous head's normalize is emitted into this head's
                # scores stream so its broadcast matmuls don't stall the PE.
                scores(0)
                for t in range(1, st_n):
                    scores(t)
                    if t == 2 and pending_norm is not None:
                        pending_norm()
                        pending_norm = None
                    pv(t - 1)
                pv(st_n - 1)
                # Evict the PV accumulator: reciprocal of the denominator row
                # into the fp16 broadcast-moving tile (row 0), and the
                # numerator rows to SBUF, freeing PSUM for the next head.
                # (reciprocal_approx_fast misreads PSUM at partition!=0 on HW,
                #  so stage the denominator row through SBUF first)
                rsb = rs128[h % 2]
                drow = npool.tile([1, s], f32, name=f"drow{h}", tag="drow")
                nc.vector.tensor_copy(drow[:], otp[hd:hd1, :])
                rrow = npool.tile([1, s], f32, name=f"rrow{h}", tag="rrow")
                nc.vector.reciprocal_approx_fast(rrow[:], drow[:])
                nc.vector.tensor_copy(rsb[0:1, :], rrow[:])
                ob = npool.tile([hd, s], f32, name=f"obuf{h}", tag="obuf")
                nc.vector.tensor_copy(ob[:], otp[0:hd, :])
                if dbg and h == 0:
                    nc.gpsimd.dma_start(dob[:], ob[:])
                    nc.gpsimd.dma_start(drs[0:1, :], rsb[0:1, :])

                # the last head's normalize gates the output projection, so
                # chunk it finer there to release early columns sooner
                cw = nf if h == h_loc - 1 else ec
                cfc = cw // nf

                def norm(ob=ob, rsb=rsb, jt=jt, off=off, h=h, cw=cw, cfc=cfc):
                    for e in range(s // cw):
                        bp = spsum.tile([hd, cw], f32, name=f"bp{h}_{e}", tag="sp")
                        for f in range(cfc):
                            q0 = e * cw + f * nf
                            nc.tensor.matmul(bp[:, f * nf:(f + 1) * nf],
                                             ones_bc[:, 0:hd], rsb[:, q0:q0 + nf],
                                             start=True, stop=True)
                        nc.vector.tensor_mul(
                            ot_sb[jt][off:off + hd, e * cw:(e + 1) * cw],
                            ob[0:hd, e * cw:(e + 1) * cw], bp[:])

                if pending_norm is not None:  # small-config fallback
                    pending_norm()
                pending_norm = norm
            pending_norm()
            if dbg:
                nc.gpsimd.dma_start(drs[1:2, :], rs128[1][0:1, :])
                nc.gpsimd.dma_start(dot[:], ot_sb[0][:])

        # ---- output projection (natural layout, fp16 writeback) ----
        with tc.tile_pool(name="fpsum", bufs=3, space="PSUM") as fpsum, \
             tc.tile_pool(name="fout", bufs=3) as fout:
            for t in range(st_n):
                po = fpsum.tile([P, d], f32, name=f"po{t}", tag="po")
                for njx in range(ndo):
                    for j in range(njt):
                        nc.tensor.matmul(
                            po[:, njx * nf:(njx + 1) * nf],
                            ot_sb[j][:, t * P:(t + 1) * P],
                            wo_r[j][:, njx * nf:(njx + 1) * nf],
                            start=(j == 0), stop=(j == njt - 1))
                ob = fout.tile([P, d], mdt, name=f"ob{t}", tag="ob")
                nc.vector.tensor_copy(ob[:], po[:])
                nc.sync.dma_start(out[t * P:(t + 1) * P, :], ob[:])

    nc.compile()
    return nc


_NC_CACHE = {}


def _get_nc():
    key = MM_DT
    if key not in _NC_CACHE:
        _NC_CACHE[key] = build_mha(mm_dt=key)
    return _NC_CACHE[key]


def build_in_maps(inputs, mm_dt=MM_DT):
    if mm_dt == "bf16":
        import ml_dtypes
        xdt = ml_dtypes.bfloat16
    else:
        xdt = np.float16

    q = np.asarray(inputs["query"], np.float32)
    k = np.asarray(inputs.get("key_", inputs.get("key")), np.float32)
    v = np.asarray(inputs["value"], np.float32)
    Wq = np.asarray(inputs["Wq"], np.float32)
    Wk = np.asarray(inputs["Wk"], np.float32)
    Wv = np.asarray(inputs["Wv"], np.float32)
    Wo = np.asarray(inputs["Wo"], np.float32)
    bq = np.asarray(inputs["bq"], np.float32)
    bk = np.asarray(inputs["bk"], np.float32)

    sc = np.float32(1.0 / np.sqrt(HD))
    qT = [np.ascontiguousarray(q[b].T).astype(xdt) for b in range(B)]
    kT = [np.ascontiguousarray(k[b].T).astype(xdt) for b in range(B)]
    vT = [np.ascontiguousarray(v[b].T).astype(xdt) for b in range(B)]
    WqT = np.ascontiguousarray(Wq.T)
    WkT = np.ascontiguousarray(Wk.T)
    WvT = np.ascontiguousarray(Wv.T)

    in_maps = []
    for core in range(N_CORES):
        b, g = divmod(core, GROUPS)
        sl = slice(g * JJ, (g + 1) * JJ)
        in_maps.append({
            "xq": qT[b],
            "xk": kT[b],
            "xv": vT[b],
            "wq": np.ascontiguousarray(WqT[:, sl]).astype(xdt),
            "wk": np.ascontiguousarray(WkT[:, sl]).astype(xdt),
            "wv": np.ascontiguousarray(WvT[:, sl]).astype(xdt),
            "wo": np.ascontiguousarray(Wo[:, sl].T).astype(xdt),
            "bqp": np.ascontiguousarray((bq[sl] * sc)[:, None]),
            "bkp": np.ascontiguousarray(bk[sl][:, None]),
        })
    return in_maps


def combine_outputs(results, inputs):
    Wo = np.asarray(inputs["Wo"], np.float32)
    bv = np.asarray(inputs["bv"], np.float32)
    bo = np.asarray(inputs["bo"], np.float32)
    const = bv @ Wo.T + bo  # exact host-side bias correction
    outp = np.empty((B, S, D), np.float32)
    for b in range(B):
        acc = results[b * GROUPS]["out"].astype(np.float32)
        for g in range(1, GROUPS):
            acc = acc + results[b * GROUPS + g]["out"].astype(np.float32)
        outp[b] = acc + const[None, :]
    return outp


def kernel(**inputs):
    import time
    from concourse.bass_utils import run_bass_kernel_spmd

    nc = _get_nc()
    in_maps = build_in_maps(inputs)
    last_err = None
    for attempt in range(3):
        try:
            res = run_bass_kernel_spmd(nc, in_maps, list(range(N_CORES)))
            return combine_outputs(res.results, inputs)
        except Exception as e:  # transient device wedge: retry
            last_err = e
            try:
                # poke each core with a trivial op to clear transient
                # exec-unit state before retrying
                import jax
                import jax.numpy as jnp
                for dvc in jax.devices()[:N_CORES]:
                    jax.device_put(jnp.zeros((8, 8)), dvc).block_until_ready()
            except Exception:
                pass
            time.sleep(5.0 * (attempt + 1))
    raise last_err


# revision 18
# speedup vs baseline: 1.5913x; 1.0246x over previous
"""Multi-head attention (B=2, S=2048, D=1024, H=16) on 8 TRN2 NeuronCores.

Sharding: core = (batch b, head-group g): 2 batches x 4 groups of 4 heads.
Each core computes its group's QKV projections, attention, and a partial
output projection; the host sums the 4 partials per batch and adds the
exact bias constant (bv @ Wo.T + bo). bq/bk are applied on device.

Engine budget per core (warm PE @2.4GHz): PE ~165us of matmul columns,
ACT ~142us of exp, DVE ~40us of evac/normalize. The kernel is structured
so the PE never idles long enough for the HAM clock gate to re-throttle:

  * every matmul presents a full 128-row stationary to the array. The
    hd=64 score matmuls are padded with explicit zero rows (per-head KT
    tiles [128, s] with zeros outside the head's 64 rows) so the padded
    rows multiply the other head's moving data by 0.0 -- same cycle
    count, full array activity.
  * projections run k-outer so each arriving x-tile is consumed once,
    back-to-back; all input DMAs are issued upfront on one queue in
    consumption order (xk, xv, xq).
  * a short burst of dummy matmuls warms the PE during the initial DMA
    window, and a dummy exp preloads the ACT table set.
  * ACT runs exp only. Projection bias+scale, PSUM evacuations, and the
    softmax normalization run on DVE (reciprocal_approx_fast on the [1,s]
    denominator row, then a PE broadcast matmul of the reciprocal).

Per-core layout:
  xT [D, S] host-transposed inputs; QT [128, S] pair-packed, KT [128, S]
  per-head zero-padded, head-dim-major so scores come out keys-on-
  partitions; the key-axis softmax reduction happens inside the P.T @ V'
  matmul via a ones-column appended to V' (PSUM row 64 of the PV output
  accumulates the softmax denominator). OT [128, S] pair-packed feeds the
  output projection as lhsT, giving the partial output in natural [S, D]
  layout, written back as fp16 (host upcasts and combines).
"""
from contextlib import ExitStack

import numpy as np

# Problem constants (hardcoded per harness contract).
B, S, D, H = 2, 2048, 1024, 16
HD = D // H          # 64
N_CORES = 8
GROUPS = N_CORES // B    # 4
H_LOC = H // GROUPS      # 4 heads per core
JJ = H_LOC * HD          # 256
P = 128

MM_DT = "fp16"  # "fp16" | "bf16"


def build_mha(s=S, d=D, h_loc=H_LOC, hd=HD, chunk=1024, nf=512, mm_dt=MM_DT,
              dbg=False):
    """Build + compile the per-core Bass program."""
    import concourse.bacc as bacc
    import concourse.tile as tile
    from concourse import mybir

    f32 = mybir.dt.float32
    _two_byte = {"bf16": mybir.dt.bfloat16, "fp16": mybir.dt.float16}
    assert mm_dt in _two_byte
    mdt = _two_byte[mm_dt]
    in_dt = mdt
    Exp = mybir.ActivationFunctionType.Exp
    MULT = mybir.AluOpType.mult
    ADD = mybir.AluOpType.add

    jj = h_loc * hd
    hd1 = hd + 1
    ktd = d // P
    njt = (jj + P - 1) // P
    st_n = s // P
    nf = min(nf, s)
    nfc = s // nf            # moving chunks per full row
    ndo = (d + nf - 1) // nf

    nc = bacc.Bacc("TRN2", target_bir_lowering=False, debug=False)

    xq = nc.dram_tensor("xq", [d, s], in_dt, kind="ExternalInput").ap()
    xk = nc.dram_tensor("xk", [d, s], in_dt, kind="ExternalInput").ap()
    xv = nc.dram_tensor("xv", [d, s], in_dt, kind="ExternalInput").ap()
    wq = nc.dram_tensor("wq", [d, jj], in_dt, kind="ExternalInput").ap()
    wk = nc.dram_tensor("wk", [d, jj], in_dt, kind="ExternalInput").ap()
    wv = nc.dram_tensor("wv", [d, jj], in_dt, kind="ExternalInput").ap()
    wo = nc.dram_tensor("wo", [jj, d], in_dt, kind="ExternalInput").ap()
    bqp = nc.dram_tensor("bqp", [jj, 1], f32, kind="ExternalInput").ap()
    bkp = nc.dram_tensor("bkp", [jj, 1], f32, kind="ExternalInput").ap()
    out = nc.dram_tensor("out", [s, d], mdt, kind="ExternalOutput").ap()
    if dbg:
        dq = nc.dram_tensor("dq", [P, s], mdt, kind="ExternalOutput").ap()
        dk = nc.dram_tensor("dk", [P, s], mdt, kind="ExternalOutput").ap()
        dv = nc.dram_tensor("dv", [P, P], mdt, kind="ExternalOutput").ap()
        dpt = nc.dram_tensor("dpt", [P, min(1024, s)], mdt, kind="ExternalOutput").ap()
        dob = nc.dram_tensor("dob", [hd, s], f32, kind="ExternalOutput").ap()
        drs = nc.dram_tensor("drs", [2, s], mdt, kind="ExternalOutput").ap()
        dot = nc.dram_tensor("dot", [P, s], mdt, kind="ExternalOutput").ap()

    with tile.TileContext(nc) as tc, ExitStack() as ctx:
        persist = ctx.enter_context(tc.tile_pool(name="persist", bufs=1))

        qt_sb = [persist.tile([P, s], mdt, name=f"qt{j}", tag=f"qt{j}") for j in range(njt)]
        # per-head KT, zero rows outside the head's hd slice (full-row scores)
        kt_sb = [persist.tile([P, s], mdt, name=f"kt{h}", tag=f"kt{h}") for h in range(h_loc)]
        ot_sb = [persist.tile([P, s], mdt, name=f"ot{j}", tag=f"ot{j}") for j in range(njt)]
        # padded per-(seq-tile, head) PV stationaries: [V_h | ones | zeros]
        v_sb = [[persist.tile([P, P], mdt, name=f"v{t}_{h}", tag=f"v{t}_{h}")
                 for h in range(h_loc)] for t in range(st_n)]
        # weights land as one wide tile each (one big DMA: per-dma_start
        # completion latency was serializing the input stream)
        wq_b = persist.tile([P, ktd * jj], mdt, name="wq_b", tag="wq_b")
        wk_b = persist.tile([P, ktd * jj], mdt, name="wk_b", tag="wk_b")
        wv_b = persist.tile([P, ktd * jj], mdt, name="wv_b", tag="wv_b")
        wo_b = persist.tile([P, njt * d], mdt, name="wo_b", tag="wo_b")
        wq_r = [wq_b[:, k * jj:(k + 1) * jj] for k in range(ktd)]
        wk_r = [wk_b[:, k * jj:(k + 1) * jj] for k in range(ktd)]
        wv_r = [wv_b[:, k * jj:(k + 1) * jj] for k in range(ktd)]
        wo_r = [wo_b[:, j * d:(j + 1) * d] for j in range(njt)]
        bq_sb = persist.tile([P, njt], f32, name="bq_sb", tag="bq_sb")
        bk_sb = persist.tile([P, njt], f32, name="bk_sb", tag="bk_sb")
        ones_v = persist.tile([P, 1], f32, name="ones_v", tag="ones_v")
        # norm broadcast: stationary row0=1 rest 0; moving row0=recip(denom)
        ones_bc = persist.tile([P, hd], mdt, name="ones_bc", tag="ones_bc")
        rs128 = [persist.tile([P, s], mdt, name=f"rs{i}", tag=f"rs{i}")
                 for i in range(2)]
        wm_a = persist.tile([P, nf], mdt, name="wm_a", tag="wm_a")
        ep_t = persist.tile([1, 8], f32, name="ep_t", tag="ep_t")

        # ---- preamble: exp-table preload, PE warmup, zero padding ----
        nc.vector.memset(ep_t[:], 0.0)
        nc.scalar.activation(ep_t[:], ep_t[:], Exp)  # pulls ACT table load early
        nc.vector.memset(ones_v[:], 1.0)
        nc.vector.memset(wm_a[:], 0.0)
        nc.gpsimd.memset(ones_bc[:], 0.0)
        nc.gpsimd.memset(ones_bc[0:1, :], 1.0)
        for i in range(2):
            nc.gpsimd.memset(rs128[i][:], 0.0)
        for h in range(h_loc):
            off = (h * hd) % P
            if off > 0:
                nc.gpsimd.memset(kt_sb[h][0:off, :], 0.0)
            if off + hd < P:
                nc.gpsimd.memset(kt_sb[h][off + hd:P, :], 0.0)
        for j in range(njt):
            nc.scalar.dma_start(bq_sb[:, j:j + 1], bqp[j * P:(j + 1) * P, :])
            nc.scalar.dma_start(bk_sb[:, j:j + 1], bkp[j * P:(j + 1) * P, :])

        # ---- weights ----
        # wk on the sync queue ahead of the x tensors (first consumer);
        # everything else on the scalar-engine HWDGE queue. One DMA each.
        nc.sync.dma_start(wk_b[:], wk.rearrange("(k p) j -> p k j", p=P))
        nc.scalar.dma_start(wv_b[:], wv.rearrange("(k p) j -> p k j", p=P))
        nc.scalar.dma_start(wq_b[:], wq.rearrange("(k p) j -> p k j", p=P))
        nc.scalar.dma_start(wo_b[:], wo.rearrange("(j p) d -> p j d", p=P))

        # PE warmup burst (runs while the first x tiles stream in)
        with tc.tile_pool(name="wup", bufs=1, space="PSUM") as wup:
            wm_p = wup.tile([P, nf], f32, name="wm_p", tag="wm_p")
            for i in range(10):
                nc.tensor.matmul(wm_p[:], wm_a[:, 0:P], wm_a[:], start=True, stop=True)
            # token reader so the warmup matmuls can't be elided
            nc.vector.tensor_copy(ep_t[0:1, 0:8], wm_p[0:1, 0:8])

        # ---- projections (K, V, Q; k-outer so each x tile is consumed once) ----
        # Each x tensor lands as one wide [128, ktd*s] tile via two DMAs
        # (halves, so compute can start on the first half). Two pool slots:
        # xq reuses xk's slot once the K projection has consumed it.
        kh = ktd // 2
        with tc.tile_pool(name="xrpool", bufs=2) as xrpool:
            def load_xr(xdr, nm):
                xb = xrpool.tile([P, ktd * s], mdt, name=f"x{nm}", tag="xbig")
                for hlf in range(2):
                    rows = slice(hlf * kh * P, (hlf + 1) * kh * P)
                    nc.sync.dma_start(
                        xb[:, hlf * kh * s:(hlf + 1) * kh * s],
                        xdr[rows, :].rearrange("(k p) s -> p k s", p=P))
                return [xb[:, k * s:(k + 1) * s] for k in range(ktd)]

            xk_t = load_xr(xk, "k")
            xv_t = load_xr(xv, "v")
            xq_t = load_xr(xq, "q")

            # K projection -> per-head zero-padded KT
            with tc.tile_pool(name="kpsum", bufs=1, space="PSUM") as kpsum:
                ppk = [kpsum.tile([P, s], f32, name=f"ppk{j}", tag=f"ppj{j}")
                       for j in range(njt)]
                for k in range(ktd):
                    for j in range(njt):
                        for c in range(nfc):
                            nc.tensor.matmul(
                                ppk[j][:, c * nf:(c + 1) * nf],
                                wk_r[k][:, j * P:(j + 1) * P],
                                xk_t[k][:, c * nf:(c + 1) * nf],
                                start=(k == 0), stop=(k == ktd - 1))
                for j in range(njt):
                    for hh in range(P // hd):
                        h = j * (P // hd) + hh
                        if h >= h_loc:
                            continue
                        r0 = hh * hd
                        for c in range(nfc):
                            nc.vector.tensor_scalar(
                                kt_sb[h][r0:r0 + hd, c * nf:(c + 1) * nf],
                                ppk[j][r0:r0 + hd, c * nf:(c + 1) * nf],
                                1.0, bk_sb[r0:r0 + hd, j:j + 1],
                                op0=MULT, op1=ADD)

            # V projection -> padded PV stationaries
            with tc.tile_pool(name="vpsum", bufs=1, space="PSUM") as vpsum:
                for t in range(st_n):
                    pv = vpsum.tile([P, jj], f32, name=f"pv{t}", tag="pv", bufs=3)
                    for k in range(ktd):
                        nc.tensor.matmul(pv[:], xv_t[k][:, t * P:(t + 1) * P],
                                         wv_r[k][:], start=(k == 0), stop=(k == ktd - 1))
                    for h in range(h_loc):
                        vt = v_sb[t][h]
                        nc.vector.tensor_copy(vt[:, 0:hd], pv[:, h * hd:(h + 1) * hd])
                        nc.vector.tensor_copy(vt[:, hd:hd1], ones_v[:])
                        if hd1 < P:
                            nc.gpsimd.memset(vt[:, hd1:P], 0.0)

            # Q projection -> pair-packed QT (scale folded via DVE)
            sc = float(1.0 / np.sqrt(hd))
            with tc.tile_pool(name="qpsum", bufs=1, space="PSUM") as qpsum:
                ppq = [qpsum.tile([P, s], f32, name=f"ppq{j}", tag=f"ppj{j}")
                       for j in range(njt)]
                for k in range(ktd):
                    for j in range(njt):
                        for c in range(nfc):
                            nc.tensor.matmul(
                                ppq[j][:, c * nf:(c + 1) * nf],
                                wq_r[k][:, j * P:(j + 1) * P],
                                xq_t[k][:, c * nf:(c + 1) * nf],
                                start=(k == 0), stop=(k == ktd - 1))
                for j in range(njt):
                    for c in range(nfc):
                        nc.vector.tensor_scalar(
                            qt_sb[j][:, c * nf:(c + 1) * nf],
                            ppq[j][:, c * nf:(c + 1) * nf],
                            sc, bq_sb[:, j:j + 1],
                            op0=MULT, op1=ADD)

        if dbg:
            nc.gpsimd.dma_start(dq[:], qt_sb[0][:])
            nc.gpsimd.dma_start(dk[:], kt_sb[0][:])
            nc.gpsimd.dma_start(dv[:], v_sb[0][0][:])

        # ---- attention ----
        # Per head, two passes over the full sequence:
        #   pass 1: scores.T tiles (full-row stationary from the padded KT)
        #           -> exp over [128, s] -> PT tiles
        #   pass 2: PV accumulation -> [128, s] psum, row hd = denominators
        # PSUM: sp [128, ec] x2 (4 banks) + otp [128, s] (4 banks).
        ec = min(1024, s)          # exp / score-psum chunk of the q axis
        nec = s // ec
        efc = ec // nf
        with tc.tile_pool(name="spsum", bufs=2, space="PSUM") as spsum, \
             tc.tile_pool(name="opsum", bufs=1, space="PSUM") as opsum, \
             tc.tile_pool(name="ptpool", bufs=3 * nec + 2) as ptpool, \
             tc.tile_pool(name="npool", bufs=2) as npool:
            pending_norm = None
            for h in range(h_loc):
                jt = (h * hd) // P
                off = (h * hd) % P
                otp = opsum.tile([P, s], f32, name=f"otp{h}", tag="otp")
                pts = {}

                def scores(t):
                    for e in range(nec):
                        sp = spsum.tile([P, ec], f32, name=f"sp{h}_{t}_{e}", tag="sp")
                        for f in range(efc):
                            q0 = e * ec + f * nf
                            nc.tensor.matmul(
                                sp[:, f * nf:(f + 1) * nf],
                                kt_sb[h][:, t * P:(t + 1) * P],
                                qt_sb[jt][:, q0:q0 + nf],
                                start=True, stop=True)
                        pt = ptpool.tile([P, ec], mdt, name=f"pt{h}_{t}_{e}", tag="pt")
                        nc.scalar.activation(pt[:], sp[:], Exp)
                        if dbg and h == 0 and t == 0 and e == 0:
                            nc.gpsimd.dma_start(dpt[:, 0:ec], pt[:])
                        pts[t, e] = pt

                def pv(t):
                    for e in range(nec):
                        for f in range(efc):
                            q0 = e * ec + f * nf
                            nc.tensor.matmul(
                                otp[:, q0:q0 + nf],
                                v_sb[t][h][:],
                                pts[t, e][:, f * nf:(f + 1) * nf],
                                start=(t == 0), stop=(t == st_n - 1))
                        del pts[t, e]

                # software-pipeline: scores(t+1) emitted before pv(t); the
                # previous head's normalize is emitted into this head's
                # scores stream so its broadcast matmuls don't stall the PE.
                scores(0)
                for t in range(1, st_n):
                    scores(t)
                    if t == 2 and pending_norm is not None:
                        pending_norm()
                        pending_norm = None
                    pv(t - 1)
                pv(st_n - 1)
                # Evict the PV accumulator (numerators first so the PSUM bank
                # frees for the next head ASAP), then the denominator row ->
                # reciprocal -> fp16 broadcast-moving tile (row 0).
                # (reciprocal_approx_fast misreads PSUM at partition!=0 on HW,
                #  so stage the denominator row through SBUF first)
                rsb = rs128[h % 2]
                ob = npool.tile([hd, s], f32, name=f"obuf{h}", tag="obuf")
                nc.vector.tensor_copy(ob[:], otp[0:hd, :])
                drow = npool.tile([1, s], f32, name=f"drow{h}", tag="drow", bufs=1)
                nc.vector.tensor_copy(drow[:], otp[hd:hd1, :])
                rrow = npool.tile([1, s], f32, name=f"rrow{h}", tag="rrow", bufs=1)
                nc.vector.reciprocal_approx_fast(rrow[:], drow[:])
                nc.vector.tensor_copy(rsb[0:1, :], rrow[:])
                if dbg and h == 0:
                    nc.gpsimd.dma_start(dob[:], ob[:])
                    nc.gpsimd.dma_start(drs[0:1, :], rsb[0:1, :])

                # the last head's normalize gates the output projection, so
                # chunk it finer there to release early columns sooner
                cw = nf if h == h_loc - 1 else ec
                cfc = cw // nf

                def norm(ob=ob, rsb=rsb, jt=jt, off=off, h=h, cw=cw, cfc=cfc):
                    for e in range(s // cw):
                        bp = spsum.tile([hd, cw], f32, name=f"bp{h}_{e}", tag="sp")
                        for f in range(cfc):
                            q0 = e * cw + f * nf
                            nc.tensor.matmul(bp[:, f * nf:(f + 1) * nf],
                                             ones_bc[:, 0:hd], rsb[:, q0:q0 + nf],
                                             start=True, stop=True)
                        nc.vector.tensor_mul(
                            ot_sb[jt][off:off + hd, e * cw:(e + 1) * cw],
                            ob[0:hd, e * cw:(e + 1) * cw], bp[:])

                if pending_norm is not None:  # small-config fallback
                    pending_norm()
                pending_norm = norm
            pending_norm()
            if dbg:
                nc.gpsimd.dma_start(drs[1:2, :], rs128[1][0:1, :])
                nc.gpsimd.dma_start(dot[:], ot_sb[0][:])

        # ---- output projection (natural layout, fp16 writeback) ----
        with tc.tile_pool(name="fpsum", bufs=3, space="PSUM") as fpsum, \
             tc.tile_pool(name="fout", bufs=3) as fout:
            for t in range(st_n):
                po = fpsum.tile([P, d], f32, name=f"po{t}", tag="po")
                for njx in range(ndo):
                    for j in range(njt):
                        nc.tensor.matmul(
                            po[:, njx * nf:(njx + 1) * nf],
                            ot_sb[j][:, t * P:(t + 1) * P],
                            wo_r[j][:, njx * nf:(njx + 1) * nf],
                            start=(j == 0), stop=(j == njt - 1))
                ob = fout.tile([P, d], mdt, name=f"ob{t}", tag="ob")
                nc.vector.tensor_copy(ob[:], po[:])
                nc.sync.dma_start(out[t * P:(t + 1) * P, :], ob[:])

    nc.compile()
    return nc


_NC_CACHE = {}


def _get_nc():
    key = MM_DT
    if key not in _NC_CACHE:
        _NC_CACHE[key] = build_mha(mm_dt=key)
    return _NC_CACHE[key]


def build_in_maps(inputs, mm_dt=MM_DT):
    if mm_dt == "bf16":
        import ml_dtypes
        xdt = ml_dtypes.bfloat16
    else:
        xdt = np.float16

    q = np.asarray(inputs["query"], np.float32)
    k = np.asarray(inputs.get("key_", inputs.get("key")), np.float32)
    v = np.asarray(inputs["value"], np.float32)
    Wq = np.asarray(inputs["Wq"], np.float32)
    Wk = np.asarray(inputs["Wk"], np.float32)
    Wv = np.asarray(inputs["Wv"], np.float32)
    Wo = np.asarray(inputs["Wo"], np.float32)
    bq = np.asarray(inputs["bq"], np.float32)
    bk = np.asarray(inputs["bk"], np.float32)

    sc = np.float32(1.0 / np.sqrt(HD))
    qT = [np.ascontiguousarray(q[b].T).astype(xdt) for b in range(B)]
    kT = [np.ascontiguousarray(k[b].T).astype(xdt) for b in range(B)]
    vT = [np.ascontiguousarray(v[b].T).astype(xdt) for b in range(B)]
    WqT = np.ascontiguousarray(Wq.T)
    WkT = np.ascontiguousarray(Wk.T)
    WvT = np.ascontiguousarray(Wv.T)

    in_maps = []
    for core in range(N_CORES):
        b, g = divmod(core, GROUPS)
        sl = slice(g * JJ, (g + 1) * JJ)
        in_maps.append({
            "xq": qT[b],
            "xk": kT[b],
            "xv": vT[b],
            "wq": np.ascontiguousarray(WqT[:, sl]).astype(xdt),
            "wk": np.ascontiguousarray(WkT[:, sl]).astype(xdt),
            "wv": np.ascontiguousarray(WvT[:, sl]).astype(xdt),
            "wo": np.ascontiguousarray(Wo[:, sl].T).astype(xdt),
            "bqp": np.ascontiguousarray((bq[sl] * sc)[:, None]),
            "bkp": np.ascontiguousarray(bk[sl][:, None]),
        })
    return in_maps


def combine_outputs(results, inputs):
    Wo = np.asarray(inputs["Wo"], np.float32)
    bv = np.asarray(inputs["bv"], np.float32)
    bo = np.asarray(inputs["bo"], np.float32)
    const = bv @ Wo.T + bo  # exact host-side bias correction
    outp = np.empty((B, S, D), np.float32)
    for b in range(B):
        acc = results[b * GROUPS]["out"].astype(np.float32)
        for g in range(1, GROUPS):
            acc = acc + results[b * GROUPS + g]["out"].astype(np.float32)
        outp[b] = acc + const[None, :]
    return outp


def kernel(**inputs):
    import time
    from concourse.bass_utils import run_bass_kernel_spmd

    nc = _get_nc()
    in_maps = build_in_maps(inputs)
    last_err = None
    for attempt in range(3):
        try:
            res = run_bass_kernel_spmd(nc, in_maps, list(range(N_CORES)))
            return combine_outputs(res.results, inputs)
        except Exception as e:  # transient device wedge: retry
            last_err = e
            try:
                # poke each core with a trivial op to clear transient
                # exec-unit state before retrying
                import jax
                import jax.numpy as jnp
                for dvc in jax.devices()[:N_CORES]:
                    jax.device_put(jnp.zeros((8, 8)), dvc).block_until_ready()
            except Exception:
                pass
            time.sleep(5.0 * (attempt + 1))
    raise last_err


# revision 24
# speedup vs baseline: 1.6474x; 1.0352x over previous
"""Multi-head attention (B=2, S=2048, D=1024, H=16) on 8 TRN2 NeuronCores.

Sharding: core = (batch b, head-group g): 2 batches x 4 groups of 4 heads.
Each core computes its group's QKV projections, attention, and a partial
output projection; the host sums the 4 partials per batch and adds the
exact bias constant (bv @ Wo.T + bo). bq/bk are applied on device.

Engine budget per core (warm PE @2.4GHz): PE ~165us of matmul columns,
ACT ~142us of exp, DVE ~40us of evac/normalize. The kernel is structured
so the PE never idles long enough for the HAM clock gate to re-throttle:

  * every matmul presents a full 128-row stationary to the array. The
    hd=64 score matmuls are padded with explicit zero rows (per-head KT
    tiles [128, s] with zeros outside the head's 64 rows) so the padded
    rows multiply the other head's moving data by 0.0 -- same cycle
    count, full array activity.
  * projections run k-outer so each arriving x-tile is consumed once,
    back-to-back; all input DMAs are issued upfront on one queue in
    consumption order (xk, xv, xq).
  * a short burst of dummy matmuls warms the PE during the initial DMA
    window, and a dummy exp preloads the ACT table set.
  * ACT runs exp only. Projection bias+scale, PSUM evacuations, and the
    softmax normalization run on DVE (reciprocal_approx_fast on the [1,s]
    denominator row, then a PE broadcast matmul of the reciprocal).

Per-core layout:
  xT [D, S] host-transposed inputs; QT [128, S] pair-packed, KT [128, S]
  per-head zero-padded, head-dim-major so scores come out keys-on-
  partitions; the key-axis softmax reduction happens inside the P.T @ V'
  matmul via a ones-column appended to V' (PSUM row 64 of the PV output
  accumulates the softmax denominator). OT [128, S] pair-packed feeds the
  output projection as lhsT, giving the partial output in natural [S, D]
  layout, written back as fp16 (host upcasts and combines).
"""
from contextlib import ExitStack

import numpy as np

# Problem constants (hardcoded per harness contract).
B, S, D, H = 2, 2048, 1024, 16
HD = D // H          # 64
N_CORES = 8
GROUPS = N_CORES // B    # 4
H_LOC = H // GROUPS      # 4 heads per core
JJ = H_LOC * HD          # 256
P = 128

MM_DT = "fp16"  # "fp16" | "bf16"


def build_mha(s=S, d=D, h_loc=H_LOC, hd=HD, chunk=1024, nf=512, mm_dt=MM_DT,
              dbg=False):
    """Build + compile the per-core Bass program."""
    import concourse.bacc as bacc
    import concourse.tile as tile
    from concourse import mybir

    f32 = mybir.dt.float32
    _two_byte = {"bf16": mybir.dt.bfloat16, "fp16": mybir.dt.float16}
    assert mm_dt in _two_byte
    mdt = _two_byte[mm_dt]
    in_dt = mdt
    Exp = mybir.ActivationFunctionType.Exp
    MULT = mybir.AluOpType.mult
    ADD = mybir.AluOpType.add

    jj = h_loc * hd
    hd1 = hd + 1
    ktd = d // P
    njt = (jj + P - 1) // P
    st_n = s // P
    nf = min(nf, s)
    nfc = s // nf            # moving chunks per full row
    ndo = (d + nf - 1) // nf

    nc = bacc.Bacc("TRN2", target_bir_lowering=False, debug=False)

    xq = nc.dram_tensor("xq", [d, s], in_dt, kind="ExternalInput").ap()
    xk = nc.dram_tensor("xk", [d, s], in_dt, kind="ExternalInput").ap()
    xv = nc.dram_tensor("xv", [d, s], in_dt, kind="ExternalInput").ap()
    wq = nc.dram_tensor("wq", [d, jj], in_dt, kind="ExternalInput").ap()
    wk = nc.dram_tensor("wk", [d, jj], in_dt, kind="ExternalInput").ap()
    wv = nc.dram_tensor("wv", [d, jj], in_dt, kind="ExternalInput").ap()
    wo = nc.dram_tensor("wo", [jj, d], in_dt, kind="ExternalInput").ap()
    bqp = nc.dram_tensor("bqp", [jj, 1], f32, kind="ExternalInput").ap()
    bkp = nc.dram_tensor("bkp", [jj, 1], f32, kind="ExternalInput").ap()
    out = nc.dram_tensor("out", [s, d], mdt, kind="ExternalOutput").ap()
    if dbg:
        dq = nc.dram_tensor("dq", [P, s], mdt, kind="ExternalOutput").ap()
        dk = nc.dram_tensor("dk", [P, s], mdt, kind="ExternalOutput").ap()
        dv = nc.dram_tensor("dv", [P, P], mdt, kind="ExternalOutput").ap()
        dpt = nc.dram_tensor("dpt", [P, min(1024, s)], mdt, kind="ExternalOutput").ap()
        dob = nc.dram_tensor("dob", [hd, s], f32, kind="ExternalOutput").ap()
        drs = nc.dram_tensor("drs", [2, s], mdt, kind="ExternalOutput").ap()
        dot = nc.dram_tensor("dot", [P, s], mdt, kind="ExternalOutput").ap()

    with tile.TileContext(nc) as tc, ExitStack() as ctx:
        persist = ctx.enter_context(tc.tile_pool(name="persist", bufs=1))

        qt_sb = [persist.tile([P, s], mdt, name=f"qt{j}", tag=f"qt{j}") for j in range(njt)]
        # per-head KT, zero rows outside the head's hd slice (full-row scores)
        kt_sb = [persist.tile([P, s], mdt, name=f"kt{h}", tag=f"kt{h}") for h in range(h_loc)]
        ot_sb = [persist.tile([P, s], mdt, name=f"ot{j}", tag=f"ot{j}") for j in range(njt)]
        # padded per-(seq-tile, head) PV stationaries: [V_h | ones | zeros]
        v_sb = [[persist.tile([P, P], mdt, name=f"v{t}_{h}", tag=f"v{t}_{h}")
                 for h in range(h_loc)] for t in range(st_n)]
        # weights land as one wide tile each (one big DMA: per-dma_start
        # completion latency was serializing the input stream)
        wq_b = persist.tile([P, ktd * jj], mdt, name="wq_b", tag="wq_b")
        wk_b = persist.tile([P, ktd * jj], mdt, name="wk_b", tag="wk_b")
        wv_b = persist.tile([P, ktd * jj], mdt, name="wv_b", tag="wv_b")
        wo_b = persist.tile([P, njt * d], mdt, name="wo_b", tag="wo_b")
        wq_r = [wq_b[:, k * jj:(k + 1) * jj] for k in range(ktd)]
        wk_r = [wk_b[:, k * jj:(k + 1) * jj] for k in range(ktd)]
        wv_r = [wv_b[:, k * jj:(k + 1) * jj] for k in range(ktd)]
        wo_r = [wo_b[:, j * d:(j + 1) * d] for j in range(njt)]
        bq_sb = persist.tile([P, njt], f32, name="bq_sb", tag="bq_sb")
        bk_sb = persist.tile([P, njt], f32, name="bk_sb", tag="bk_sb")
        ones_v = persist.tile([P, 1], f32, name="ones_v", tag="ones_v")
        # norm broadcast: stationary row0=1 rest 0; moving row0=recip(denom)
        ones_bc = persist.tile([P, hd], mdt, name="ones_bc", tag="ones_bc")
        rs128 = [persist.tile([P, s], mdt, name=f"rs{i}", tag=f"rs{i}")
                 for i in range(2)]
        wm_a = persist.tile([P, nf], mdt, name="wm_a", tag="wm_a")
        ep_t = persist.tile([1, 8], f32, name="ep_t", tag="ep_t")

        # ---- preamble: exp-table preload, PE warmup, zero padding ----
        nc.vector.memset(ep_t[:], 0.0)
        nc.scalar.activation(ep_t[:], ep_t[:], Exp)  # pulls ACT table load early
        nc.vector.memset(ones_v[:], 1.0)
        nc.vector.memset(wm_a[:], 0.0)
        nc.gpsimd.memset(ones_bc[:], 0.0)
        nc.gpsimd.memset(ones_bc[0:1, :], 1.0)
        for i in range(2):
            nc.gpsimd.memset(rs128[i][:], 0.0)
        for h in range(h_loc):
            off = (h * hd) % P
            if off > 0:
                nc.gpsimd.memset(kt_sb[h][0:off, :], 0.0)
            if off + hd < P:
                nc.gpsimd.memset(kt_sb[h][off + hd:P, :], 0.0)
        for j in range(njt):
            nc.scalar.dma_start(bq_sb[:, j:j + 1], bqp[j * P:(j + 1) * P, :])
            nc.scalar.dma_start(bk_sb[:, j:j + 1], bkp[j * P:(j + 1) * P, :])

        # ---- weights ----
        # Two HWDGE queues run in parallel: sync carries wk+xk+xq, scalar
        # carries wv+xv+wq+wo (xv DMAs are emitted in load_xr below).
        nc.sync.dma_start(wk_b[:], wk.rearrange("(k p) j -> p k j", p=P))
        nc.scalar.dma_start(wv_b[:], wv.rearrange("(k p) j -> p k j", p=P))

        # PE warmup burst (runs while the first x tiles stream in)
        with tc.tile_pool(name="wup", bufs=1, space="PSUM") as wup:
            wm_p = wup.tile([P, nf], f32, name="wm_p", tag="wm_p")
            for i in range(10):
                nc.tensor.matmul(wm_p[:], wm_a[:, 0:P], wm_a[:], start=True, stop=True)
            # token reader so the warmup matmuls can't be elided
            nc.vector.tensor_copy(ep_t[0:1, 0:8], wm_p[0:1, 0:8])

        # ---- projections (K, V, Q; k-outer so each x tile is consumed once) ----
        # Each x tensor lands as one wide [128, ktd*s] tile via two DMAs
        # (halves, so compute can start on the first half). Two pool slots:
        # xq reuses xk's slot once the K projection has consumed it.
        kh = ktd // 2
        with tc.tile_pool(name="xrpool", bufs=2) as xrpool:
            def load_xr(xdr, nm, eng):
                xb = xrpool.tile([P, ktd * s], mdt, name=f"x{nm}", tag="xbig")
                for hlf in range(2):
                    rows = slice(hlf * kh * P, (hlf + 1) * kh * P)
                    eng.dma_start(
                        xb[:, hlf * kh * s:(hlf + 1) * kh * s],
                        xdr[rows, :].rearrange("(k p) s -> p k s", p=P))
                return [xb[:, k * s:(k + 1) * s] for k in range(ktd)]

            xk_t = load_xr(xk, "k", nc.sync)
            xv_t = load_xr(xv, "v", nc.scalar)
            xq_t = load_xr(xq, "q", nc.sync)
            nc.scalar.dma_start(wq_b[:], wq.rearrange("(k p) j -> p k j", p=P))
            nc.scalar.dma_start(wo_b[:], wo.rearrange("(j p) d -> p j d", p=P))

            # K projection -> per-head zero-padded KT
            with tc.tile_pool(name="kpsum", bufs=1, space="PSUM") as kpsum:
                ppk = [kpsum.tile([P, s], f32, name=f"ppk{j}", tag=f"ppj{j}")
                       for j in range(njt)]
                for k in range(ktd):
                    for j in range(njt):
                        for c in range(nfc):
                            nc.tensor.matmul(
                                ppk[j][:, c * nf:(c + 1) * nf],
                                wk_r[k][:, j * P:(j + 1) * P],
                                xk_t[k][:, c * nf:(c + 1) * nf],
                                start=(k == 0), stop=(k == ktd - 1))
                for j in range(njt):
                    for hh in range(P // hd):
                        h = j * (P // hd) + hh
                        if h >= h_loc:
                            continue
                        r0 = hh * hd
                        for c in range(nfc):
                            nc.vector.tensor_scalar(
                                kt_sb[h][r0:r0 + hd, c * nf:(c + 1) * nf],
                                ppk[j][r0:r0 + hd, c * nf:(c + 1) * nf],
                                1.0, bk_sb[r0:r0 + hd, j:j + 1],
                                op0=MULT, op1=ADD)

            # V projection -> padded PV stationaries
            with tc.tile_pool(name="vpsum", bufs=1, space="PSUM") as vpsum:
                for t in range(st_n):
                    pv = vpsum.tile([P, jj], f32, name=f"pv{t}", tag="pv", bufs=3)
                    for k in range(ktd):
                        nc.tensor.matmul(pv[:], xv_t[k][:, t * P:(t + 1) * P],
                                         wv_r[k][:], start=(k == 0), stop=(k == ktd - 1))
                    for h in range(h_loc):
                        vt = v_sb[t][h]
                        nc.vector.tensor_copy(vt[:, 0:hd], pv[:, h * hd:(h + 1) * hd])
                        nc.vector.tensor_copy(vt[:, hd:hd1], ones_v[:])
                        if hd1 < P:
                            nc.gpsimd.memset(vt[:, hd1:P], 0.0)

            # Q projection -> pair-packed QT (scale folded via DVE)
            sc = float(1.0 / np.sqrt(hd))
            with tc.tile_pool(name="qpsum", bufs=1, space="PSUM") as qpsum:
                ppq = [qpsum.tile([P, s], f32, name=f"ppq{j}", tag=f"ppj{j}")
                       for j in range(njt)]
                for k in range(ktd):
                    for j in range(njt):
                        for c in range(nfc):
                            nc.tensor.matmul(
                                ppq[j][:, c * nf:(c + 1) * nf],
                                wq_r[k][:, j * P:(j + 1) * P],
                                xq_t[k][:, c * nf:(c + 1) * nf],
                                start=(k == 0), stop=(k == ktd - 1))
                for j in range(njt):
                    for c in range(nfc):
                        nc.vector.tensor_scalar(
                            qt_sb[j][:, c * nf:(c + 1) * nf],
                            ppq[j][:, c * nf:(c + 1) * nf],
                            sc, bq_sb[:, j:j + 1],
                            op0=MULT, op1=ADD)

        if dbg:
            nc.gpsimd.dma_start(dq[:], qt_sb[0][:])
            nc.gpsimd.dma_start(dk[:], kt_sb[0][:])
            nc.gpsimd.dma_start(dv[:], v_sb[0][0][:])

        # ---- attention ----
        # Per head, two passes over the full sequence:
        #   pass 1: scores.T tiles (full-row stationary from the padded KT)
        #           -> exp over [128, s] -> PT tiles
        #   pass 2: PV accumulation -> [128, s] psum, row hd = denominators
        # PSUM: sp [128, ec] x2 (4 banks) + otp [128, s] (4 banks).
        ec = min(1024, s)          # exp / score-psum chunk of the q axis
        nec = s // ec
        efc = ec // nf
        with tc.tile_pool(name="spsum", bufs=2, space="PSUM") as spsum, \
             tc.tile_pool(name="ptpool", bufs=3 * nec + 2) as ptpool, \
             tc.tile_pool(name="npool", bufs=2) as npool:
            pts = {}
            otps = {}
            obs = {}
            opsum_ctx = ExitStack()
            opsum = opsum_ctx.enter_context(
                tc.tile_pool(name="opsum", bufs=1, space="PSUM"))

            def scores(h, t):
                for e in range(nec):
                    sp = spsum.tile([P, ec], f32, name=f"sp{h}_{t}_{e}", tag="sp")
                    for f in range(efc):
                        q0 = e * ec + f * nf
                        nc.tensor.matmul(
                            sp[:, f * nf:(f + 1) * nf],
                            kt_sb[h][:, t * P:(t + 1) * P],
                            qt_sb[(h * hd) // P][:, q0:q0 + nf],
                            start=True, stop=True)
                    pt = ptpool.tile([P, ec], mdt, name=f"pt{h}_{t}_{e}", tag="pt")
                    nc.scalar.activation(pt[:], sp[:], Exp)
                    if dbg and h == 0 and t == 0 and e == 0:
                        nc.gpsimd.dma_start(dpt[:, 0:ec], pt[:])
                    pts[h, t, e] = pt

            def pv(h, t):
                if t == 0:
                    otps[h] = opsum.tile([P, s], f32, name=f"otp{h}", tag="otp")
                otp = otps[h]
                for e in range(nec):
                    pt = pts.pop((h, t, e))
                    for f in range(efc):
                        q0 = e * ec + f * nf
                        nc.tensor.matmul(
                            otp[:, q0:q0 + nf],
                            v_sb[t][h][:],
                            pt[:, f * nf:(f + 1) * nf],
                            start=(t == 0), stop=(t == st_n - 1))

            def evac(h, dve_drow=True):
                # Evict the PV accumulator (numerators first so the PSUM
                # banks free ASAP), then denominator row -> approx recip ->
                # fp16 broadcast-moving tile row 0.
                # (reciprocal_approx_fast misreads PSUM at partition!=0 on
                #  HW, so the denominator row is staged through SBUF.)
                otp = otps.pop(h)
                ob = npool.tile([hd, s], f32, name=f"obuf{h}", tag="obuf")
                nc.vector.tensor_copy(ob[:], otp[0:hd, :])
                obs[h] = ob
                drow = npool.tile([1, s], f32, name=f"drow{h}", tag="drow", bufs=1)
                if dve_drow:
                    nc.vector.tensor_copy(drow[:], otp[hd:hd1, :])
                else:  # last head: ACT is idle by now, DVE is the gate
                    nc.scalar.copy(drow[:], otp[hd:hd1, :])
                rrow = npool.tile([1, s], f32, name=f"rrow{h}", tag="rrow", bufs=1)
                nc.vector.reciprocal_approx_fast(rrow[:], drow[:])
                nc.vector.tensor_copy(rs128[h % 2][0:1, :], rrow[:])
                if dbg and h == 0:
                    nc.gpsimd.dma_start(dob[:], ob[:])
                    nc.gpsimd.dma_start(drs[0:1, :], rs128[0][0:1, :])

            def norm_chunk(h, e, cw, cfc):
                ob, rsb = obs[h], rs128[h % 2]
                jt, off = (h * hd) // P, (h * hd) % P
                bp = spsum.tile([hd, cw], f32, name=f"bp{h}_{e}", tag="sp")
                for f in range(cfc):
                    q0 = e * cw + f * nf
                    nc.tensor.matmul(bp[:, f * nf:(f + 1) * nf],
                                     ones_bc[:, 0:hd], rsb[:, q0:q0 + nf],
                                     start=True, stop=True)
                nc.vector.tensor_mul(
                    ot_sb[jt][off:off + hd, e * cw:(e + 1) * cw],
                    ob[0:hd, e * cw:(e + 1) * cw], bp[:])

            def norm(h):
                for e in range(s // ec):
                    norm_chunk(h, e, ec, efc)

            # flat (h, t) software pipeline: the scores/exp stream runs
            # `lead` slots ahead of the PV stream and flows straight across
            # head boundaries, so the ACT engine (the attention bottleneck)
            # never drains. Head h's normalize is emitted two slots into
            # head h+1's stream; the last head's normalize interleaves with
            # the output projection below.
            slots = [(h, t) for h in range(h_loc) for t in range(st_n)]
            lead = 2
            norm_q = []
            for i in range(len(slots) + lead):
                if i < len(slots):
                    scores(*slots[i])
                j = i - lead
                if j < 0:
                    continue
                h2, t2 = slots[j]
                if t2 == 1 and norm_q:
                    norm(norm_q.pop(0))
                pv(h2, t2)
                if t2 == st_n - 1:
                    last = h2 == h_loc - 1
                    evac(h2, dve_drow=not last)
                    if not last:
                        norm_q.append(h2)
            assert not norm_q
            opsum_ctx.close()  # release the 4 otp banks for the out-proj

            if dbg:
                nc.gpsimd.dma_start(drs[1:2, :], rs128[1][0:1, :])

            # ---- output projection, interleaved with the last head's
            # normalize (chunk e covers seq-tiles 4e..4e+3) ----
            lh = h_loc - 1
            ncw = nf
            with tc.tile_pool(name="fpsum", bufs=2, space="PSUM") as fpsum, \
                 tc.tile_pool(name="fout", bufs=3) as fout:
                for t in range(st_n):
                    if t % (ncw // P) == 0:
                        norm_chunk(lh, t // (ncw // P), ncw, ncw // nf)
                    po = fpsum.tile([P, d], f32, name=f"po{t}", tag="po")
                    for njx in range(ndo):
                        for j in range(njt):
                            nc.tensor.matmul(
                                po[:, njx * nf:(njx + 1) * nf],
                                ot_sb[j][:, t * P:(t + 1) * P],
                                wo_r[j][:, njx * nf:(njx + 1) * nf],
                                start=(j == 0), stop=(j == njt - 1))
                    ob = fout.tile([P, d], mdt, name=f"ob{t}", tag="ob")
                    nc.vector.tensor_copy(ob[:], po[:])
                    nc.sync.dma_start(out[t * P:(t + 1) * P, :], ob[:])
            if dbg:
                nc.gpsimd.dma_start(dot[:], ot_sb[0][:])

    nc.compile()
    return nc


_NC_CACHE = {}


def _get_nc():
    key = MM_DT
    if key not in _NC_CACHE:
        _NC_CACHE[key] = build_mha(mm_dt=key)
    return _NC_CACHE[key]


def build_in_maps(inputs, mm_dt=MM_DT):
    if mm_dt == "bf16":
        import ml_dtypes
        xdt = ml_dtypes.bfloat16
    else:
        xdt = np.float16

    q = np.asarray(inputs["query"], np.float32)
    k = np.asarray(inputs.get("key_", inputs.get("key")), np.float32)
    v = np.asarray(inputs["value"], np.float32)
    Wq = np.asarray(inputs["Wq"], np.float32)
    Wk = np.asarray(inputs["Wk"], np.float32)
    Wv = np.asarray(inputs["Wv"], np.float32)
    Wo = np.asarray(inputs["Wo"], np.float32)
    bq = np.asarray(inputs["bq"], np.float32)
    bk = np.asarray(inputs["bk"], np.float32)

    sc = np.float32(1.0 / np.sqrt(HD))
    qT = [np.ascontiguousarray(q[b].T).astype(xdt) for b in range(B)]
    kT = [np.ascontiguousarray(k[b].T).astype(xdt) for b in range(B)]
    vT = [np.ascontiguousarray(v[b].T).astype(xdt) for b in range(B)]
    WqT = np.ascontiguousarray(Wq.T)
    WkT = np.ascontiguousarray(Wk.T)
    WvT = np.ascontiguousarray(Wv.T)

    in_maps = []
    for core in range(N_CORES):
        b, g = divmod(core, GROUPS)
        sl = slice(g * JJ, (g + 1) * JJ)
        in_maps.append({
            "xq": qT[b],
            "xk": kT[b],
            "xv": vT[b],
            "wq": np.ascontiguousarray(WqT[:, sl]).astype(xdt),
            "wk": np.ascontiguousarray(WkT[:, sl]).astype(xdt),
            "wv": np.ascontiguousarray(WvT[:, sl]).astype(xdt),
            "wo": np.ascontiguousarray(Wo[:, sl].T).astype(xdt),
            "bqp": np.ascontiguousarray((bq[sl] * sc)[:, None]),
            "bkp": np.ascontiguousarray(bk[sl][:, None]),
        })
    return in_maps


def combine_outputs(results, inputs):
    Wo = np.asarray(inputs["Wo"], np.float32)
    bv = np.asarray(inputs["bv"], np.float32)
    bo = np.asarray(inputs["bo"], np.float32)
    const = bv @ Wo.T + bo  # exact host-side bias correction
    outp = np.empty((B, S, D), np.float32)
    for b in range(B):
        acc = results[b * GROUPS]["out"].astype(np.float32)
        for g in range(1, GROUPS):
            acc = acc + results[b * GROUPS + g]["out"].astype(np.float32)
        outp[b] = acc + const[None, :]
    return outp


def kernel(**inputs):
    import time
    from concourse.bass_utils import run_bass_kernel_spmd

    nc = _get_nc()
    in_maps = build_in_maps(inputs)
    last_err = None
    for attempt in range(3):
        try:
            res = run_bass_kernel_spmd(nc, in_maps, list(range(N_CORES)))
            return combine_outputs(res.results, inputs)
        except Exception as e:  # transient device wedge: retry
            last_err = e
            try:
                # poke each core with a trivial op to clear transient
                # exec-unit state before retrying
                import jax
                import jax.numpy as jnp
                for dvc in jax.devices()[:N_CORES]:
                    jax.device_put(jnp.zeros((8, 8)), dvc).block_until_ready()
            except Exception:
                pass
            time.sleep(5.0 * (attempt + 1))
    raise last_err
